# revision 25
# baseline (speedup 1.0000x reference)
"""Segment-reduce contrastive loss kernel for Trainium2 (8 NeuronCores).

Strategy (data-parallel over batch, per sharding hint):
  - Each of the 8 cores gets one batch element (fs/ft: [512, 16384] f32).
  - On-device per core: per-class channel sums for features_s/features_t
    computed as one-hot matmuls on the tensor engine. Features arrive
    channel-major, so each [128pix x 128ch] block is PE-transposed first
    (pixels must sit on the partition/contraction dim).
  - Per-class partial sums [19, 512] x2 are DMA'd out; the host sums the
    8 cores' partials (the "all-reduce"), computes counts, normalizes and
    does the tiny 19x19 contrastive logsumexp in numpy.

Performance notes (best measured ~205us; run-to-run spread ~205-222us):
  - The binding constraint is the tensor engine (~179us busy, >99% dense
    once started): per 128-px group it runs 8 fp32 transposes (~117ns)
    plus 2 fp32r segment matmuls (~250ns). The DMA stream (~64 MiB at
    ~400 GB/s aggregate) has ~10% headroom over PE.
  - Segment matmuls run in float32r (1 cycle/row vs fp32's 4); the
    PSUM->SBUF copy doubles as the required fp32r rounding op (the BIR
    verifier rejects fp32r matmul inputs straight from DMA).
  - Matmuls are emitted a few groups behind their transposes so the
    in-order PE never stalls on the DVE/ACT copy chain.
  - All feature DMAs dispatch on the sync/SP HWDGE ring. Putting them on
    the scalar/ACT ring serializes them behind the per-group copies
    (measured large regression); SWDGE (gpsimd) aggravates the slow
    DMA-engine-15 straggler.
  - The final 1024 px land via TWO dma_starts (768+256) into one tile:
    subtile deps let its first 6 groups compute while the 256-px tail is
    in flight, so almost no compute remains after the last byte. Keeping
    the end of the stream to few, small DMAs also avoids completion-sem
    lane-recycling stalls (8 lanes, round-robin) that otherwise idle all
    16 DMA engines for ~5-9us before the last chunks dispatch.
  - PE starts ~16.5us in (first pair + consts + pstate ramp) with ~2
    chunks of backlog and stays dense. Starting PE earlier (small first
    chunks, or consts-first plus p-state-warming dummy transposes) DOES
    move PE start to ~10us but consumes the backlog: PE consumption
    (~11.5us/1024-px pair) nearly equals DMA delivery (~10.5us), so the
    buffer never rebuilds, every DMA hiccup becomes a PE gap, p-state
    drops inflate PE busy to ~187us, and the total measures WORSE.
"""

import sys

for _p in ("/opt/trn_rl_repo",):
    if _p not in sys.path:
        sys.path.insert(0, _p)

from contextlib import ExitStack

import numpy as np

import concourse.bass as bass
import concourse.mybir as mybir
from concourse import bacc, tile
from concourse.bass_utils import run_bass_kernel_spmd

NUM_CLASSES = 19
TEMP = 0.1
EPS = 1e-12

B, C, H, W = 8, 512, 128, 128
HW = H * W
N_CORES = 8
P = 128
F32 = mybir.dt.float32
F32R = mybir.dt.float32r

# Chunk schedule (pixels per chunk). 1024-px middle chunks keep DMA
# completion granularity fine enough for PE to stay in lockstep; the end
# taper keeps the post-last-byte compute tail tiny.
SIZES = [512, 512] + [1024] * 15
MAIN_PIX = 1024  # slot size of the main nat ring
END_PIX = 1024   # slot size of the taper tile (last chunk)
N_END = 1
# The last chunk lands via TWO dma_starts (768 + 256 px) into one tile:
# subtile deps let its first 6 groups compute while the 256-px tail is
# still in flight, so almost no compute remains after the last byte.
TAPER_SPLIT = 768


def build_nc(C_=C, HW_=HW):
    NCH = C_ // P        # channel blocks
    NG = HW_ // P        # pixel groups of 128
    assert sum(SIZES) == HW_ and all(s % P == 0 for s in SIZES)

    nc = bacc.Bacc()
    fs = nc.declare_dram_parameter("fs", [C_, HW_], F32, isOutput=False)
    ft = nc.declare_dram_parameter("ft", [C_, HW_], F32, isOutput=False)
    # misc: [identity 128 | iota 19 | labT NG] packed along the free dim so
    # the consts arrive in ONE DMA (multiple DMA-completion sems on one
    # consumer instruction overflow walrus's per-instruction sync slots).
    misc = nc.declare_dram_parameter("misc", [P, P + NUM_CLASSES + NG], F32, isOutput=False)
    out = nc.declare_dram_parameter("sums", [NUM_CLASSES, 2 * C_], F32, isOutput=True)

    srcs = {"s": fs, "t": ft}
    dma_eng = {}

    with ExitStack() as ctx:
        tc = ctx.enter_context(tile.TileContext(nc))
        const_pool = ctx.enter_context(tc.tile_pool(name="const", bufs=1))
        nat_pool = ctx.enter_context(tc.tile_pool(name="nat", bufs=4))
        natE_pool = ctx.enter_context(tc.tile_pool(name="natE", bufs=1))
        psumT_pool = ctx.enter_context(tc.tile_pool(name="psumT", bufs=3, space="PSUM"))
        acc_pool = ctx.enter_context(tc.tile_pool(name="acc", bufs=1, space="PSUM"))
        sbT_pool = ctx.enter_context(tc.tile_pool(name="sbT", bufs=5))
        oh_pool = ctx.enter_context(tc.tile_pool(name="oh", bufs=6))
        outp_pool = ctx.enter_context(tc.tile_pool(name="outp", bufs=1))

        dma_eng = {"s": nc.sync, "t": nc.sync}

        acc = {
            t: acc_pool.tile([P, C_], F32, tag=f"acc_{t}", name=f"acc_{t}")
            for t in ("s", "t")
        }

        pend = []

        def _mm(item):
            # fp32r matmuls reject non-zero col-group tile_position, so all
            # groups accumulate into partition rows 0..18 of each bank; at
            # 1 cycle/row the lost sub-array concurrency is cheap.
            g, t, oh, sT = item
            nc.tensor.matmul(
                acc[t][0:NUM_CLASSES, :],
                oh[:],
                sT[:],
                start=(g == 0),
                stop=(g == NG - 1),
            )

        misc_sb = None
        ident = iota = lab_sb = None

        pix0 = 0
        g = 0
        for j, size in enumerate(SIZES):
            is_end = j >= len(SIZES) - N_END
            pool = natE_pool if is_end else nat_pool
            slot_pix = END_PIX if is_end else MAIN_PIX
            tagsfx = "E" if is_end else ""
            nat = {}
            for t in ("s", "t"):
                # One DMA per tensor per chunk: all 4 channel blocks in a
                # single 3D access pattern (fewer triggers/sems, bigger
                # descriptor batches per queue).
                nt = pool.tile(
                    [P, NCH * size],
                    F32,
                    tag=f"nat{tagsfx}_{t}",
                    name=f"nat_{t}_{j}",
                    padded_shape=[P, NCH * slot_pix],
                )
                if is_end:
                    for w0, w1 in ((0, TAPER_SPLIT), (TAPER_SPLIT, size)):
                        dma_eng[t].dma_start(
                            nt[:].rearrange("p (k w) -> p k w", k=NCH)[:, :, w0:w1],
                            srcs[t].rearrange("(k p) w -> p k w", p=P)[
                                :, :, pix0 + w0 : pix0 + w1
                            ],
                        )
                else:
                    dma_eng[t].dma_start(
                        nt[:].rearrange("p (k w) -> p k w", k=NCH),
                        srcs[t].rearrange("(k p) w -> p k w", p=P)[:, :, pix0 : pix0 + size],
                    )
                nat[t] = nt
                if j == 0 and t == "t":
                    # Consts ride behind the first feature pair; they land
                    # long before the first compute needs them, and the
                    # ring's first dispatch stays the s1 chunk.
                    misc_sb = const_pool.tile(
                        [P, P + NUM_CLASSES + NG], F32, tag="misc", name="misc_sb"
                    )
                    nc.sync.dma_start(misc_sb[:], misc[:])
                    ident = misc_sb[:, 0:P]
                    iota = misc_sb[:, P : P + NUM_CLASSES]
                    lab_sb = misc_sb[:, P + NUM_CLASSES : P + NUM_CLASSES + NG]
                    # Warm-up transpose reading only the const tile:
                    # pre-pays the misc DMA wait on PE, so the first real
                    # transpose needs just one wait (walrus allows a single
                    # embedded sync-wait per instruction).
                    warm = psumT_pool.tile([P, P], F32, tag="pT_s", name="warm")
                    nc.tensor.transpose(warm[:, 0:P], ident, ident)
            for gl in range(size // P):
                oh = oh_pool.tile([P, NUM_CLASSES], F32R, tag="oh")
                nc.vector.tensor_scalar(
                    oh[:], iota, lab_sb[:, g : g + 1], None, mybir.AluOpType.is_equal
                )
                for t in ("s", "t"):
                    pT = psumT_pool.tile([P, C_], F32, tag=f"pT_{t}")
                    for k in range(NCH):
                        nc.tensor.transpose(
                            pT[:, k * P : (k + 1) * P],
                            nat[t][:, k * size + gl * P : k * size + (gl + 1) * P],
                            ident,
                        )
                    # fp32r output: rounds for the fp32r segment matmul
                    # (1 cycle/row vs fp32's 4).
                    sT = sbT_pool.tile([P, C_], F32R, tag=f"sT_{t}")
                    if t == "s":
                        nc.vector.tensor_copy(sT[:], pT[:])
                    else:
                        nc.scalar.copy(sT[:], pT[:])
                    pend.append((g, t, oh, sT))
                # Emit segment matmuls a few groups late so the in-order PE
                # can run group g+1's transposes while group g's PSUM->SBUF
                # copies complete (otherwise every matmul stalls on its copy).
                while len(pend) > 4:
                    _mm(pend.pop(0))
                g += 1
            pix0 += size
        # Drain the trailing matmuls s-first: acc_s finishes several
        # matmuls early, so its output copy and DMA overlap the remaining
        # t matmuls and the t-half copy instead of serializing after them.
        for item in [it for it in pend if it[1] == "s"]:
            _mm(item)
        ob = outp_pool.tile([NUM_CLASSES, 2 * C_], F32, tag="ob", name="ob")
        nc.vector.tensor_copy(ob[:, 0:C_], acc["s"][0:NUM_CLASSES, :])
        nc.sync.dma_start(out[:, 0:C_], ob[:, 0:C_])
        for item in [it for it in pend if it[1] == "t"]:
            _mm(item)
        pend.clear()
        nc.scalar.copy(ob[:, C_ : 2 * C_], acc["t"][0:NUM_CLASSES, :])
        nc.sync.dma_start(out[:, C_ : 2 * C_], ob[:, C_ : 2 * C_])
    nc.finalize()
    return nc


_NC_CACHE = None


def _get_nc():
    global _NC_CACHE
    if _NC_CACHE is None:
        _NC_CACHE = build_nc()
    return _NC_CACHE


def make_misc(lab_flat, ng):
    """[identity 128 | iota 19 | labT ng] packed along the free dim."""
    labT = lab_flat.reshape(ng, P).T.astype(np.float32)
    iota = np.tile(np.arange(NUM_CLASSES, dtype=np.float32), (P, 1))
    return np.ascontiguousarray(
        np.concatenate([np.eye(P, dtype=np.float32), iota, labT], axis=1)
    )


def _make_in_maps(features_s, features_t, labels):
    in_maps = []
    for i in range(N_CORES):
        in_maps.append(
            {
                "fs": np.ascontiguousarray(features_s[i].reshape(C, HW)),
                "ft": np.ascontiguousarray(features_t[i].reshape(C, HW)),
                "misc": make_misc(labels[i].reshape(-1), HW // P),
            }
        )
    return in_maps


def _finish_on_host(results, labels):
    S_s = np.zeros((NUM_CLASSES, C), np.float64)
    S_t = np.zeros((NUM_CLASSES, C), np.float64)
    for r in results:
        S_s += r["sums"][:, 0:C]
        S_t += r["sums"][:, C : 2 * C]
    counts = np.bincount(
        labels.reshape(-1), minlength=NUM_CLASSES
    ).astype(np.float64)
    denom = np.maximum(counts, 1.0)[:, None]

    def l2n(x):
        n = np.linalg.norm(x, axis=1, keepdims=True)
        return x / np.maximum(n, EPS)

    logits = (l2n(S_s / denom) @ l2n(S_t / denom).T) / TEMP
    m = logits.max(axis=1, keepdims=True)
    lse = m[:, 0] + np.log(np.exp(logits - m).sum(axis=1))
    per_class = np.diag(logits) - lse
    present = counts > 0
    loss = -np.sum(np.where(present, per_class, 0.0)) / np.sum(present)
    return np.asarray(loss, dtype=np.float32)


def kernel(features_s, features_t, labels, _trace=False):
    features_s = np.asarray(features_s, dtype=np.float32)
    features_t = np.asarray(features_t, dtype=np.float32)
    labels = np.asarray(labels)
    nc = _get_nc()
    in_maps = _make_in_maps(features_s, features_t, labels)
    res = run_bass_kernel_spmd(nc, in_maps, list(range(N_CORES)), trace=_trace)
    loss = _finish_on_host(res.results, labels)
    if _trace:
        return loss, res
    return loss
